# revision 2
# baseline (speedup 1.0000x reference)
"""Trainium2 Bass kernel for nn_LFVSSMBlockV66 (B=4, C=128, H=W=64).

Single launch on 4 cores, one full batch per core.  All weights are baked
into the NEFF as inline constants (loaded to HBM once at model-load time),
so the only per-call tunnel traffic is x (fp8_e3m4, 0.5 MB/core) in and
the scaled residual delta (fp8_e3m4, 0.5 MB/core) out.  The host applies
the final residual add in fp32: out = x + delta / OUT_SCALE.

Device program = the previous half-split launch-1 program with the
channel-half loop unrolled (two halves of 80 scan channels processed
sequentially) plus the SE-attention tail merged in (the old launch 2).
dBu is formed inside the scan as (G5@delta)*(G5@u)*BP -- the G5 gather is
a row-copy, so gather(delta*u) == gather(delta)*gather(u); this avoids
materialising delta*u at [160 x L].

Runner: a cached jax.jit of shard_map(bass_exec) over a fixed 4-device
mesh; the output donor buffer is cycled call-to-call (the kernel fully
writes its output, so donor contents are irrelevant), avoiding the
per-call upload of zero buffers.
"""
import sys, os
sys.path.insert(0, '/opt/trn_rl_repo')

import hashlib
import numpy as np
import ml_dtypes
from contextlib import ExitStack

from concourse import bass, mybir, tile

fp32 = mybir.dt.float32
bf16 = mybir.dt.bfloat16
f8 = mybir.dt.float8e3
AF = mybir.ActivationFunctionType
OP = mybir.AluOpType

B_, C_, H_, W_ = 4, 128, 64, 64
L = H_ * W_                      # 4096
DIN, N, DTR = 160, 24, 8
DH = DIN // 2                    # 80 per half
NT = 16                          # d-tiles of 5 per half
G = C_ // 4                      # 32
CH = 512                         # phase-A chunk (one psum bank)
CHS = 1024                       # scan chunk
NCH = L // CH                    # 8
NCHS = L // CHS                  # 4
EPS = 1e-5
NCORE = 4
OUT_SCALE = 16.0

bf = ml_dtypes.bfloat16
f8np = ml_dtypes.float8_e3m4

# This container's walrus rejects >1 sync wait per instruction; split the
# extras onto NoOps.
_ws_ctr = [0]


def split_excess_waits(nc, max_waits=1):
    for fn in nc.m.functions:
        for blk in fn.blocks:
            out, changed = [], False
            for inst in blk.instructions:
                si = getattr(inst, 'sync_info', None)
                waits = list(si.on_wait) if si is not None and si.on_wait else []
                if len(waits) > max_waits:
                    for w in waits[:-max_waits]:
                        nop = mybir.InstNoOp(name=f"I-ws{_ws_ctr[0]}", ins=[], outs=[])
                        _ws_ctr[0] += 1
                        nop.engine = inst.engine
                        nop.sync_info = mybir.SyncInfo(on_wait=[w], on_update=[])
                        out.append(nop)
                    inst.sync_info = mybir.SyncInfo(
                        on_wait=waits[-max_waits:], on_update=list(si.on_update))
                    changed = True
                out.append(inst)
            if changed:
                blk.instructions = out


def _seq_views(ap2d):
    """Per-group seq-order read views of a (128, 4096) C-major spatial AP:
    v_g[c, l] = x[32g + c, pi_g(l)].  Partition-aligned (view g lives on
    partitions 32g..32g+31)."""
    v0 = ap2d[0:G, :]
    v1 = ap2d[G:2 * G, :][:, ::-1]
    v2 = ap2d[2 * G:3 * G, :].rearrange('p (h w) -> p h w', h=64).transpose([0, 2, 1])
    v3 = ap2d[3 * G:4 * G, :].rearrange('p (h w) -> p h w', h=64).transpose([0, 2, 1])[:, ::-1, ::-1]
    return [v0, v1, v2, v3]


def _chunk(view, c0, csz):
    if view.ndim == 2:
        return view[:, c0:c0 + csz]
    rows = view.shape[2]
    return view[:, c0 // rows:(c0 + csz) // rows, :]


def _f3(ap):
    """(p, csz) -> (p, csz//64, 64) to shape-match 3D chunk views."""
    return ap.rearrange('p (a b) -> p a b', b=64)


def _prep_weights(inputs):
    """Host-side inline-constant tensors (identical on all cores)."""
    bfc = lambda a: np.ascontiguousarray(np.asarray(a, dtype=np.float32)).astype(bf)
    f32c = lambda a: np.ascontiguousarray(np.asarray(a, dtype=np.float32))
    w = {}
    w["pre_g"] = f32c(inputs["pre_gamma"]).reshape(C_, 1)
    w["pre_b"] = f32c(inputs["pre_beta"]).reshape(C_, 1)
    w["gb_g"] = f32c(inputs["gb_norm_gamma"]).reshape(C_, 1)
    w["gb_b"] = f32c(inputs["gb_norm_beta"]).reshape(C_, 1)
    w["ones1"] = bfc(np.ones((1, C_)))
    w["o128"] = bfc(np.full((C_, 1), 1.0 / C_))
    w["o32"] = bfc(np.full((G, 1), 1.0 / C_))
    w["epsv"] = np.full((C_, 1), EPS, np.float32)
    w["onesr"] = bfc(np.ones((1, CH)))
    w["conv1T"] = bfc(np.asarray(inputs["lb_conv1_w"]).T)
    dwall = np.concatenate([np.asarray(inputs["lb_dw1_w"]),
                            np.asarray(inputs["lb_dw2_w"]),
                            np.asarray(inputs["lb_dw3_w"])], axis=0)
    dw9 = np.zeros((96, 9 * 96), np.float32)
    for k in range(9):
        dw9[np.arange(96), k * 96 + np.arange(96)] = dwall[:, k // 3, k % 3]
    w["dw9T"] = bfc(dw9)
    pwt = np.asarray(inputs["lb_pw_w"]).astype(np.float32).T    # (128, 128)
    w["pwAT"] = bfc(pwt[0:G, :])
    w["pwBT"] = bfc(pwt[G:, :])
    fuse = np.asarray(inputs["fuse_w"]).astype(np.float32)      # (128, 256)
    w["fuseLT"] = bfc(fuse[:, :C_].T)
    w["fuseXT"] = bfc((fuse[:, :C_] + fuse[:, C_:]).T)
    gbs = float(np.asarray(inputs["gb_scale"]).reshape(-1)[0])
    w["w2T"] = bfc((gbs * fuse[:, C_:]).T)
    fusT = np.asarray(inputs["gb_fusion_w"]).T
    w["fusGT"] = bfc(np.concatenate([fusT[gi * G:(gi + 1) * G, :]
                                     for gi in range(4)], axis=1))
    inw = np.asarray(inputs["m_in_proj_w"]).astype(np.float32)  # (320, 128)
    w["inpT"] = bfc(inw.T)                                      # (128, 320)
    cw = np.asarray(inputs["m_conv_w"]).astype(np.float32)      # (160, 4)
    cb = f32c(inputs["m_conv_b"])
    xp = np.asarray(inputs["m_x_proj_w"]).astype(np.float32)    # (56, 160)
    dtw = np.asarray(inputs["m_dt_w"]).astype(np.float32)       # (160, 8)
    A = -np.exp(np.asarray(inputs["m_A_log"], dtype=np.float32))  # (160, 24)
    ow = np.asarray(inputs["m_out_proj_w"]).astype(np.float32)  # (128, 160)
    Dp = f32c(inputs["m_D"])
    dtb = f32c(inputs["m_dt_b"])
    for h in range(2):
        o0 = h * DH
        convd = np.zeros((DH, 4 * DH), np.float32)
        for k in range(4):
            convd[np.arange(DH), k * DH + np.arange(DH)] = cw[o0:o0 + DH, k]
        w[f"convdT{h}"] = bfc(convd)
        w[f"convbT{h}"] = bfc(cb[o0:o0 + DH].reshape(1, DH))
        w[f"xpT{h}"] = bfc(xp[:, o0:o0 + DH].T)
        w[f"dtT{h}"] = bfc(dtw[o0:o0 + DH, :].T)
        w[f"dt_b{h}"] = dtb[o0:o0 + DH].reshape(DH, 1)
        A_P = np.zeros((120, NT), np.float32)
        for t in range(NT):
            for n in range(N):
                for j in range(5):
                    A_P[n * 5 + j, t] = A[o0 + t * 5 + j, n]
        w[f"A_P{h}"] = A_P
        w[f"outT{h}"] = bfc(ow[:, o0:o0 + DH].T)
        w[f"D80_{h}"] = Dp[o0:o0 + DH].reshape(DH, 1)
    R24m = np.zeros((N, 120), np.float32)
    for n in range(N):
        R24m[n, n * 5:(n + 1) * 5] = 1.0
    w["R24"] = bfc(R24m)
    G5a = np.zeros((DH, NT * 120), np.float32)
    S = np.zeros((120, NT * DH), np.float32)
    for t in range(NT):
        for n in range(N):
            for j in range(5):
                G5a[t * 5 + j, t * 120 + n * 5 + j] = 1.0
                S[n * 5 + j, t * DH + t * 5 + j] = 1.0
    w["G5all"] = bfc(G5a)
    w["S_all"] = bfc(S)
    w["fc1T"] = bfc((np.asarray(inputs["att_fc1_w"], dtype=np.float32) / L).T)
    w["b1"] = f32c(inputs["att_fc1_b"]).reshape(16, 1)
    w["fc2T"] = bfc(np.asarray(inputs["att_fc2_w"]).T)
    w["b2"] = f32c(inputs["att_fc2_b"]).reshape(C_, 1)
    rs = float(np.asarray(inputs["res_scale"]).reshape(-1)[0])
    w["resv"] = np.full((C_, 1), rs * OUT_SCALE, np.float32)
    return w


def build_full(wts, debug=False):
    nc = bass.Bass()
    x_in = nc.declare_dram_parameter("x", [C_, L], f8, isOutput=False)
    o_out = nc.declare_dram_parameter("o", [C_, L], f8, isOutput=True)
    W = {k: nc.inline_tensor(v, name=f"c_{k}") for k, v in wts.items()}
    dbg = {}
    if debug:
        P = nc.declare_dram_parameter
        for nm, shp, dt in [("d_xnb", [C_, L], bf16), ("d_seqC", [C_, L], bf16),
                            ("d_u", [2 * DH, L], bf16), ("d_dblb", [56, L], bf16),
                            ("d_del", [2 * DH, L], bf16), ("d_y3", [2 * DH, L], bf16),
                            ("d_osb", [C_, L], bf16), ("d_fgb", [C_, L], bf16),
                            ("d_local", [C_, L], bf16), ("d_fused", [C_, L], bf16),
                            ("d_pool", [C_, 1], fp32)]:
            dbg[nm] = P(nm, shp, dt, isOutput=True)

    with tile.TileContext(nc) as tc, ExitStack() as ctx:
        wp = ctx.enter_context(tc.tile_pool(name="wp", bufs=1))
        pp = ctx.enter_context(tc.tile_pool(name="pp", bufs=1))
        s2 = ctx.enter_context(tc.tile_pool(name="s2", bufs=2))

        def load(key, shape, dt):
            t = wp.tile(shape, dt, tag=f"w_{key}", name=f"w_{key}")
            nc.sync.dma_start(t[:], W[key][:])
            return t

        w_pre_g = load("pre_g", [C_, 1], fp32)
        w_pre_b = load("pre_b", [C_, 1], fp32)
        w_gb_g = load("gb_g", [C_, 1], fp32)
        w_gb_b = load("gb_b", [C_, 1], fp32)
        w_ones1 = load("ones1", [1, C_], bf16)
        w_o128 = load("o128", [C_, 1], bf16)
        w_o32 = load("o32", [G, 1], bf16)
        w_epsv = load("epsv", [C_, 1], fp32)
        w_onesr = load("onesr", [1, CH], bf16)
        w_conv1T = load("conv1T", [G, G], bf16)
        w_dw9T = load("dw9T", [96, 9 * 96], bf16)
        w_pwAT = load("pwAT", [G, C_], bf16)
        w_pwBT = load("pwBT", [96, C_], bf16)
        w_fuseLT = load("fuseLT", [C_, C_], bf16)
        w_fuseXT = load("fuseXT", [C_, C_], bf16)
        w_w2T = load("w2T", [C_, C_], bf16)
        w_fusGT = load("fusGT", [G, 4 * C_], bf16)
        w_inpT = load("inpT", [C_, 320], bf16)
        w_convdT = [load(f"convdT{h}", [DH, 4 * DH], bf16) for h in range(2)]
        w_convbT = [load(f"convbT{h}", [1, DH], bf16) for h in range(2)]
        w_xpT = [load(f"xpT{h}", [DH, 56], bf16) for h in range(2)]
        w_dtT = [load(f"dtT{h}", [DTR, DH], bf16) for h in range(2)]
        w_dt_b = [load(f"dt_b{h}", [DH, 1], fp32) for h in range(2)]
        w_A_P = [load(f"A_P{h}", [120, NT], fp32) for h in range(2)]
        w_outT = [load(f"outT{h}", [DH, C_], bf16) for h in range(2)]
        w_D80 = [load(f"D80_{h}", [DH, 1], fp32) for h in range(2)]
        w_R24 = load("R24", [N, 120], bf16)
        w_G5all = load("G5all", [DH, NT * 120], bf16)
        w_S = load("S_all", [120, NT * DH], bf16)
        w_fc1T = load("fc1T", [C_, 16], bf16)
        w_b1 = load("b1", [16, 1], fp32)
        w_fc2T = load("fc2T", [16, C_], bf16)
        w_b2 = load("b2", [C_, 1], fp32)
        w_resv = load("resv", [C_, 1], fp32)

        def ln_stats(row_pairs, rp):
            """row_pairs(kind, c0) -> [(lhsT_ap, rhs_ap)] accumulated into a
            (1, CH) stat psum.  Returns (mu_row, rs_row) (1, L) bf16 tiles."""
            murow = rp.tile([1, L], bf16, tag="murow")
            s2row = rp.tile([1, L], bf16, tag="s2row")
            rows = {'mu': murow, 's2': s2row}
            with tc.tile_pool(name="st_ps", bufs=2, space="PSUM") as stp:
                for c0 in range(0, L, CH):
                    for kind in ('mu', 's2'):
                        ps_t = stp.tile([1, CH], fp32, tag=f"ps_{kind}")
                        pairs = row_pairs(kind, c0)
                        for i, (lh, rh) in enumerate(pairs):
                            nc.tensor.matmul(ps_t[:], lh, rh, start=(i == 0),
                                             stop=(i == len(pairs) - 1))
                        nc.scalar.copy(rows[kind][:, c0:c0 + CH], ps_t[:])
            mup = s2.tile([C_, 32], bf16, tag="mup")
            nc.sync.dma_start(mup[:], murow[:])
            s2p = s2.tile([C_, 32], bf16, tag="s2p")
            nc.sync.dma_start(s2p[:], s2row[:])
            musq = s2.tile([C_, 32], fp32, tag="musq")
            nc.scalar.square(musq[:], mup[:])
            var = s2.tile([C_, 32], fp32, tag="var")
            nc.vector.tensor_sub(var[:], s2p[:], musq[:])
            sd = s2.tile([C_, 32], fp32, tag="sd")
            nc.scalar.activation(sd[:], var[:], AF.Sqrt, bias=w_epsv[:])
            rsp = s2.tile([C_, 32], fp32, tag="rsp")
            nc.vector.reciprocal(rsp[:], sd[:])
            rsbp = s2.tile([C_, 32], bf16, tag="rsbp")
            nc.vector.tensor_copy(rsbp[:], rsp[:])
            rsr = rp.tile([1, L], bf16, tag="rsr")
            nc.sync.dma_start(rsr[:], rsbp[:])
            return murow, rsr

        # ---------------- pre-LN ----------------
        xnb = pp.tile([C_, L], bf16, tag="xnb")
        localb = pp.tile([C_, L], bf16, tag="localb")
        with tc.tile_pool(name="ph1", bufs=1) as p1:
            x8t = p1.tile([C_, L], f8, tag="x8t")
            nc.sync.dma_start(x8t[:], x_in[:])
            xCb = p1.tile([C_, L], bf16, tag="xCb")
            nc.scalar.copy(xCb[:], x8t[:])

            def pre_rows(kind, c0):
                if kind == 'mu':
                    return [(w_o128[:], xCb[:, c0:c0 + CH])]
                sqs = s2.tile([C_, CH], bf16, tag="sqsP")
                nc.scalar.square(sqs[:], xCb[:, c0:c0 + CH])
                return [(w_o128[:], sqs[:])]

            mur, rsr = ln_stats(pre_rows, p1)
            with tc.tile_pool(name="bc_ps", bufs=2, space="PSUM") as bcp:
                for c0 in range(0, L, CH):
                    muP = bcp.tile([C_, CH], fp32, tag="muP")
                    nc.tensor.matmul(muP[:], w_ones1[:], mur[:, c0:c0 + CH],
                                     start=True, stop=True)
                    rsP = bcp.tile([C_, CH], fp32, tag="rsP")
                    nc.tensor.matmul(rsP[:], w_ones1[:], rsr[:, c0:c0 + CH],
                                     start=True, stop=True)
                    t1 = s2.tile([C_, CH], fp32, tag="t1")
                    nc.vector.tensor_sub(t1[:], xCb[:, c0:c0 + CH], muP[:])
                    nc.vector.tensor_mul(t1[:], t1[:], rsP[:])
                    nc.vector.tensor_scalar(xnb[:, c0:c0 + CH], t1[:], w_pre_g[:],
                                            w_pre_b[:], OP.mult, OP.add)
        if debug:
            nc.sync.dma_start(dbg["d_xnb"][:], xnb[:])

        # ---------------- local branch ----------------
        with tc.tile_pool(name="ph2", bufs=1) as p2:
            pad0 = p2.tile([96, 66 * 66], bf16, tag="pad0")
            nc.vector.memset(pad0[:], 0.0)
            pad0v = pad0[:].rearrange('p (r c) -> p r c', r=66)
            nc.sync.dma_start(pad0v[:, 1:65, 1:65],
                              xnb[G:, :].rearrange('p (h w) -> p h w', h=64))
            y_a = p2.tile([G, L], bf16, tag="y_a")
            y_bb = p2.tile([96, L], bf16, tag="y_bb")
            with tc.tile_pool(name="lb_ps", bufs=2, space="PSUM") as lbp:
                for c0 in range(0, L, CH):
                    r0 = c0 // 64
                    y32 = lbp.tile([G, CH], fp32, tag="y32")
                    nc.tensor.matmul(y32[:], w_conv1T[:], xnb[0:G, c0:c0 + CH],
                                     start=True, stop=True)
                    nc.scalar.copy(y_a[:, c0:c0 + CH], y32[:])
                    y96 = lbp.tile([96, CH], fp32, tag="y96")
                    for k in range(9):
                        ky, kx = k // 3, k % 3
                        rhs = pad0v[:, ky + r0:ky + r0 + 8, kx:kx + 64]
                        nc.tensor.matmul(y96[:], w_dw9T[:, k * 96:(k + 1) * 96],
                                         rhs, start=(k == 0), stop=(k == 8))
                    nc.scalar.copy(y_bb[:, c0:c0 + CH], y96[:])
                for c0 in range(0, L, CH):
                    pw_ps = lbp.tile([C_, CH], fp32, tag="pw_ps")
                    nc.tensor.matmul(pw_ps[:], w_pwAT[:], y_a[:, c0:c0 + CH],
                                     start=True, stop=False)
                    nc.tensor.matmul(pw_ps[:], w_pwBT[:], y_bb[:, c0:c0 + CH],
                                     start=False, stop=True)
                    lr1 = s2.tile([C_, CH], bf16, tag="lr1")
                    nc.vector.tensor_scalar(lr1[:], pw_ps[:], 0.1, None, OP.mult)
                    nc.vector.tensor_tensor(localb[:, c0:c0 + CH], pw_ps[:], lr1[:],
                                            OP.max)
        if debug:
            nc.sync.dma_start(dbg["d_local"][:], localb[:])

        with tc.tile_pool(name="p_tail", bufs=1) as p_tail, \
             tc.tile_pool(name="p_y3", bufs=1) as p_y3:
          with tc.tile_pool(name="p_scan", bufs=1) as p_scan:
            # ---------------- gb-LN + seq build ----------------
            with tc.tile_pool(name="p_seq", bufs=1) as p_seq:
                seqC = p_seq.tile([C_, L], bf16, tag="seqC")
                with tc.tile_pool(name="ph3", bufs=1) as p3:
                    xn1 = p3.tile([G, L], bf16, tag="xn1")
                    nc.sync.dma_start(xn1[:], xnb[G:2 * G, :])
                    xn2 = p3.tile([G, L], bf16, tag="xn2")
                    nc.sync.dma_start(xn2[:], xnb[2 * G:3 * G, :])
                    xn3 = p3.tile([G, L], bf16, tag="xn3")
                    nc.sync.dma_start(xn3[:], xnb[3 * G:, :])

                    def g_view(t, gi):
                        if gi == 1:
                            return t[:][:, ::-1]
                        v = t[:].rearrange('p (h w) -> p h w', h=64).transpose([0, 2, 1])
                        return v if gi == 2 else v[:, ::-1, ::-1]

                    # copies (partition 0-31 based) for the squares / stats
                    xnv = [xnb[0:G, :]] + [g_view(t, gi + 1)
                                           for gi, t in enumerate((xn1, xn2, xn3))]

                    def gb_rows(kind, c0):
                        if kind == 'mu':
                            return [(w_o32[:], _chunk(xnv[gi], c0, CH))
                                    for gi in range(4)]
                        pairs = []
                        for gi in range(4):
                            sqs = s2.tile([G, CH], bf16, tag="sqsP")
                            srcv = _chunk(xnv[gi], c0, CH)
                            nc.scalar.square(
                                _f3(sqs[:]) if srcv.ndim == 3 else sqs[:], srcv)
                            pairs.append((w_o32[:], sqs[:]))
                        return pairs

                    mur2, rsr2 = ln_stats(gb_rows, p3)

                    # partition-aligned direct views for the normalisation
                    xnv_t = _seq_views(xnb[:])
                    with tc.tile_pool(name="bc2_ps", bufs=2, space="PSUM") as bcp:
                        for c0 in range(0, L, CH):
                            muP = bcp.tile([C_, CH], fp32, tag="muP2")
                            nc.tensor.matmul(muP[:], w_ones1[:], mur2[:, c0:c0 + CH],
                                             start=True, stop=True)
                            rsP = bcp.tile([C_, CH], fp32, tag="rsP2")
                            nc.tensor.matmul(rsP[:], w_ones1[:], rsr2[:, c0:c0 + CH],
                                             start=True, stop=True)
                            tg = s2.tile([C_, CH], fp32, tag="tg")
                            for gi in range(4):
                                srcv = _chunk(xnv_t[gi], c0, CH)
                                sl = slice(gi * G, (gi + 1) * G)
                                if srcv.ndim == 3:
                                    nc.vector.tensor_sub(_f3(tg[sl, :]), srcv,
                                                         _f3(muP[sl, :]))
                                else:
                                    nc.vector.tensor_sub(tg[sl, :], srcv, muP[sl, :])
                                nc.vector.tensor_mul(tg[sl, :], tg[sl, :], rsP[sl, :])
                            nc.vector.tensor_scalar(seqC[:, c0:c0 + CH], tg[:],
                                                    w_gb_g[:], w_gb_b[:],
                                                    OP.mult, OP.add)
                if debug:
                    nc.sync.dma_start(dbg["d_seqC"][:], seqC[:])

                # ------------- in_proj + conv1d + silu -------------
                zs = [p_scan.tile([DH, L], bf16, tag=f"zs{h}", name=f"zs{h}")
                      for h in range(2)]
                u_t = [p_scan.tile([DH, L], bf16, tag=f"u{h}", name=f"u{h}")
                       for h in range(2)]
                xr_pad = [p_seq.tile([DH, L + 3], bf16, tag=f"xrp{h}",
                                     name=f"xrp{h}") for h in range(2)]
                for h in range(2):
                    nc.vector.memset(xr_pad[h][:, 0:3], 0.0)
                with tc.tile_pool(name="ip_ps", bufs=2, space="PSUM") as ipp:
                    for c0 in range(0, L, CH):
                        for h in range(2):
                            xr_ps = ipp.tile([DH, CH], fp32, tag=f"xr_ps{h}",
                                             name=f"xr_ps{h}")
                            nc.tensor.matmul(xr_ps[:], w_inpT[:, h * DH:(h + 1) * DH],
                                             seqC[:, c0:c0 + CH], start=True, stop=True)
                            nc.scalar.copy(xr_pad[h][:, 3 + c0:3 + c0 + CH], xr_ps[:])
                            z_ps = ipp.tile([DH, CH], fp32, tag=f"z_ps{h}",
                                            name=f"z_ps{h}")
                            nc.tensor.matmul(z_ps[:],
                                             w_inpT[:, 160 + h * DH:160 + (h + 1) * DH],
                                             seqC[:, c0:c0 + CH], start=True, stop=True)
                            zsg = s2.tile([DH, CH], bf16, tag="zsg")
                            nc.scalar.activation(zsg[:], z_ps[:], AF.Sigmoid)
                            nc.vector.tensor_mul(zs[h][:, c0:c0 + CH], zsg[:], z_ps[:])

                with tc.tile_pool(name="cv_ps", bufs=2, space="PSUM") as cvp:
                    for c0 in range(0, L, CH):
                        for h in range(2):
                            cv_ps = cvp.tile([DH, CH], fp32, tag="cv_ps")
                            for k in range(4):
                                nc.tensor.matmul(
                                    cv_ps[:],
                                    w_convdT[h][:, k * DH:(k + 1) * DH],
                                    xr_pad[h][:, c0 + k:c0 + k + CH],
                                    start=(k == 0), stop=False)
                            nc.tensor.matmul(cv_ps[:], w_convbT[h][:], w_onesr[:],
                                             start=False, stop=True)
                            usg2 = s2.tile([DH, CH], bf16, tag="usg2")
                            nc.scalar.activation(usg2[:], cv_ps[:], AF.Sigmoid)
                            nc.vector.tensor_mul(u_t[h][:, c0:c0 + CH], usg2[:],
                                                 cv_ps[:])
            if debug:
                nc.sync.dma_start(dbg["d_u"][0:DH, :], u_t[0][:])
                nc.sync.dma_start(dbg["d_u"][DH:, :], u_t[1][:])

            # ------------- x_proj + delta + B/C spread -------------
            del_t = [p_scan.tile([DH, L], bf16, tag=f"del{h}", name=f"del{h}")
                     for h in range(2)]
            BP = p_scan.tile([120, L], bf16, tag="BP")
            CPt = p_scan.tile([120, L], bf16, tag="CPt")
            with tc.tile_pool(name="p_dbc", bufs=1) as p_dbc:
                dtc = p_dbc.tile([DTR, L], bf16, tag="dtc")
                Bc = p_dbc.tile([N, L], bf16, tag="Bc")
                Cc = p_dbc.tile([N, L], bf16, tag="Cc")
                with tc.tile_pool(name="xp_ps", bufs=2, space="PSUM") as xpp:
                    for c0 in range(0, L, CH):
                        for nm, dst, lo, hi in (("dt_o", dtc, 0, DTR),
                                                ("b_o", Bc, DTR, DTR + N),
                                                ("c_o", Cc, DTR + N, 56)):
                            o_ps = xpp.tile([hi - lo, CH], fp32, tag=nm, name=nm)
                            for h in range(2):
                                nc.tensor.matmul(
                                    o_ps[:], w_xpT[h][:, lo:hi],
                                    u_t[h][:, c0:c0 + CH],
                                    start=(h == 0), stop=(h == 1))
                            nc.scalar.copy(dst[:, c0:c0 + CH], o_ps[:])
                if debug:
                    nc.sync.dma_start(dbg["d_dblb"][0:DTR, :], dtc[:])
                    nc.sync.dma_start(dbg["d_dblb"][DTR:DTR + N, :], Bc[:])
                    nc.sync.dma_start(dbg["d_dblb"][DTR + N:, :], Cc[:])

                # delta per half: softplus via exp+ln
                e80 = p_dbc.tile([DH, L], bf16, tag="e80")
                for h in range(2):
                    with tc.tile_pool(name="dt_ps", bufs=2, space="PSUM") as dtp:
                        for c0 in range(0, L, CH):
                            dt_ps = dtp.tile([DH, CH], fp32, tag="dt_ps")
                            nc.tensor.matmul(dt_ps[:], w_dtT[h][:],
                                             dtc[:, c0:c0 + CH],
                                             start=True, stop=True)
                            nc.scalar.activation(e80[:, c0:c0 + CH], dt_ps[:],
                                                 AF.Exp, bias=w_dt_b[h][:])
                    nc.scalar.activation(del_t[h][:], e80[:], AF.Ln, bias=1.0)
                if debug:
                    nc.sync.dma_start(dbg["d_del"][0:DH, :], del_t[0][:])
                    nc.sync.dma_start(dbg["d_del"][DH:, :], del_t[1][:])

                with tc.tile_pool(name="bc3_ps", bufs=2, space="PSUM") as bcp:
                    for c0 in range(0, L, CH):
                        bp_ps = bcp.tile([120, CH], fp32, tag="bp_ps")
                        nc.tensor.matmul(bp_ps[:], w_R24[:], Bc[:, c0:c0 + CH],
                                         start=True, stop=True)
                        nc.scalar.copy(BP[:, c0:c0 + CH], bp_ps[:])
                        cp_ps = bcp.tile([120, CH], fp32, tag="cp_ps")
                        nc.tensor.matmul(cp_ps[:], w_R24[:], Cc[:, c0:c0 + CH],
                                         start=True, stop=True)
                        nc.scalar.copy(CPt[:, c0:c0 + CH], cp_ps[:])

            # ---------------- selective scan (both halves) ----------------
            y3 = [p_y3.tile([DH, L], bf16, tag=f"y3_{h}", name=f"y3_{h}")
                  for h in range(2)]
            hstate = pp.tile([120, 2 * NT], bf16, tag="hstate")
            with tc.tile_pool(name="sc_ps", bufs=2, space="PSUM") as scp, \
                 tc.tile_pool(name="scu_ps", bufs=1, space="PSUM") as scup, \
                 tc.tile_pool(name="scy_ps", bufs=1, space="PSUM") as scyp, \
                 tc.tile_pool(name="scs", bufs=2) as scs:
                for h in range(2):
                    for ci in range(NCHS):
                        c0 = ci * CHS
                        y_ps = scyp.tile([DH, CHS], fp32, tag="y_ps")
                        for t in range(NT):
                            ts_ = h * NT + t
                            dP = scp.tile([120, CHS], fp32, tag="dP")
                            for s in range(2):
                                nc.tensor.matmul(
                                    dP[:, s * CH:(s + 1) * CH],
                                    w_G5all[:, t * 120:(t + 1) * 120],
                                    del_t[h][:, c0 + s * CH:c0 + (s + 1) * CH],
                                    start=True, stop=True)
                            dA = scs.tile([120, CHS], fp32, tag="dA")
                            nc.scalar.activation(dA[:], dP[:], AF.Exp,
                                                 scale=w_A_P[h][:, t:t + 1])
                            uP = scup.tile([120, CHS], fp32, tag="uP")
                            for s in range(2):
                                nc.tensor.matmul(
                                    uP[:, s * CH:(s + 1) * CH],
                                    w_G5all[:, t * 120:(t + 1) * 120],
                                    u_t[h][:, c0 + s * CH:c0 + (s + 1) * CH],
                                    start=True, stop=True)
                            uB = scs.tile([120, CHS], bf16, tag="uB")
                            nc.vector.tensor_mul(uB[:], uP[:], BP[:, c0:c0 + CHS])
                            dBu = scs.tile([120, CHS], bf16, tag="dBu")
                            nc.vector.tensor_mul(dBu[:], uB[:], dP[:])
                            hh = scs.tile([120, CHS], bf16, tag="hh")
                            init = 0.0 if ci == 0 else hstate[:, ts_:ts_ + 1]
                            nc.vector.tensor_tensor_scan(hh[:], dA[:], dBu[:], init,
                                                         OP.mult, OP.add)
                            nc.vector.tensor_copy(hstate[:, ts_:ts_ + 1],
                                                  hh[:, CHS - 1:CHS])
                            hC = scs.tile([120, CHS], bf16, tag="hC")
                            nc.vector.tensor_mul(hC[:], hh[:], CPt[:, c0:c0 + CHS])
                            for s in range(2):
                                nc.tensor.matmul(y_ps[:, s * CH:(s + 1) * CH],
                                                 w_S[:, t * DH:(t + 1) * DH],
                                                 hC[:, s * CH:(s + 1) * CH],
                                                 start=(t == 0), stop=(t == NT - 1))
                        y2 = scs.tile([DH, CHS], bf16, tag="y2")
                        nc.vector.scalar_tensor_tensor(y2[:], u_t[h][:, c0:c0 + CHS],
                                                       w_D80[h][:], y_ps[:],
                                                       OP.mult, OP.add)
                        nc.vector.tensor_mul(y3[h][:, c0:c0 + CHS], y2[:],
                                             zs[h][:, c0:c0 + CHS])
            if debug:
                nc.sync.dma_start(dbg["d_y3"][0:DH, :], y3[0][:])
                nc.sync.dma_start(dbg["d_y3"][DH:, :], y3[1][:])
          # ---------- out_proj (p_scan closed; y3 + osb alive) ----------
          osb = p_tail.tile([C_, L], bf16, tag="osb")
          with tc.tile_pool(name="op_ps", bufs=2, space="PSUM") as opp:
              for c0 in range(0, L, CH):
                  os_ps = opp.tile([C_, CH], fp32, tag="os_ps")
                  nc.tensor.matmul(os_ps[:], w_outT[0][:], y3[0][:, c0:c0 + CH],
                                   start=True, stop=False)
                  nc.tensor.matmul(os_ps[:], w_outT[1][:], y3[1][:, c0:c0 + CH],
                                   start=False, stop=True)
                  nc.scalar.copy(osb[:, c0:c0 + CH], os_ps[:])
          if debug:
              nc.sync.dma_start(dbg["d_osb"][:], osb[:])

          # ---------- un-scan + fusion ----------
        # (indent note: still inside the p_tail/p_y3 with-block)
        if True:
            fgb = p_tail.tile([C_, L], bf16, tag="fgb")
            with tc.tile_pool(name="ph6", bufs=1) as p6:
                os1c = p6.tile([G, L], bf16, tag="os1c")
                nc.sync.dma_start(os1c[:], osb[G:2 * G, :])
                os2c = p6.tile([G, L], bf16, tag="os2c")
                nc.sync.dma_start(os2c[:], osb[2 * G:3 * G, :])
                os3 = p6.tile([G, L], bf16, tag="os3")
                nc.sync.dma_start(os3[:], osb[3 * G:, :])

                def r_view(t, gi):
                    if gi == 1:
                        return t[:][:, ::-1]
                    v = t[:].rearrange('p (w h) -> p w h', w=64).transpose([0, 2, 1])
                    return v if gi == 2 else v[:, ::-1, ::-1]

                rvs = [osb[0:G, :], r_view(os1c, 1), r_view(os2c, 2), r_view(os3, 3)]
                with tc.tile_pool(name="fg_ps", bufs=2, space="PSUM") as fgp:
                    for c0 in range(0, L, CH):
                        fg_ps = fgp.tile([C_, CH], fp32, tag="fg_ps")
                        for gi in range(4):
                            nc.tensor.matmul(fg_ps[:],
                                             w_fusGT[:, gi * C_:(gi + 1) * C_],
                                             _chunk(rvs[gi], c0, CH),
                                             start=(gi == 0), stop=(gi == 3))
                        nc.scalar.copy(fgb[:, c0:c0 + CH], fg_ps[:])
            if debug:
                nc.sync.dma_start(dbg["d_fgb"][:], fgb[:])

            # ---------- fuse + pool + SE + output ----------
            fusedb = p_tail.tile([C_, L], bf16, tag="fusedb")
            poolacc = pp.tile([C_, NCH], fp32, tag="poolacc")
            with tc.tile_pool(name="fu_ps", bufs=2, space="PSUM") as fup:
                for idx, c0 in enumerate(range(0, L, CH)):
                    fu_ps = fup.tile([C_, CH], fp32, tag="fu_ps")
                    nc.tensor.matmul(fu_ps[:], w_fuseLT[:], localb[:, c0:c0 + CH],
                                     start=True, stop=False)
                    nc.tensor.matmul(fu_ps[:], w_fuseXT[:], xnb[:, c0:c0 + CH],
                                     start=False, stop=False)
                    nc.tensor.matmul(fu_ps[:], w_w2T[:], fgb[:, c0:c0 + CH],
                                     start=False, stop=True)
                    nc.scalar.activation(fusedb[:, c0:c0 + CH], fu_ps[:], AF.Copy,
                                         accum_out=poolacc[:, idx:idx + 1])
            if debug:
                nc.sync.dma_start(dbg["d_fused"][:], fusedb[:])

            poolp = pp.tile([C_, 1], fp32, tag="poolp")
            nc.vector.tensor_reduce(poolp[:], poolacc[:], mybir.AxisListType.X,
                                    OP.add)
            if debug:
                nc.sync.dma_start(dbg["d_pool"][:], poolp[:])
            poolb = pp.tile([C_, 1], bf16, tag="poolb")
            nc.vector.tensor_copy(poolb[:], poolp[:])
            with tc.tile_pool(name="se_ps", bufs=1, space="PSUM") as sep:
                h1 = sep.tile([16, 1], fp32, tag="h1")
                nc.tensor.matmul(h1[:], w_fc1T[:], poolb[:], start=True, stop=True)
                r1 = pp.tile([16, 1], bf16, tag="r1")
                nc.scalar.activation(r1[:], h1[:], AF.Relu, bias=w_b1[:])
                a_ps = sep.tile([C_, 1], fp32, tag="a_ps")
                nc.tensor.matmul(a_ps[:], w_fc2T[:], r1[:], start=True, stop=True)
                a_t = pp.tile([C_, 1], fp32, tag="a_t")
                nc.scalar.activation(a_t[:], a_ps[:], AF.Sigmoid, bias=w_b2[:])
            s_t = pp.tile([C_, 1], fp32, tag="s_t")
            nc.vector.tensor_mul(s_t[:], a_t[:], w_resv[:])
            o8 = p_tail.tile([C_, L], f8, tag="o8")
            nc.vector.tensor_scalar(o8[:], fusedb[:], s_t[:], None, OP.mult)
            nc.sync.dma_start(o_out[:], o8[:])
    return nc


# ---------------------------------------------------------------------------
_cache = {}

_WKEYS = ["pre_gamma", "pre_beta", "lb_conv1_w", "lb_dw1_w", "lb_dw2_w",
          "lb_dw3_w", "lb_pw_w", "gb_norm_gamma", "gb_norm_beta",
          "gb_fusion_w", "gb_scale", "m_in_proj_w", "m_conv_w", "m_conv_b",
          "m_x_proj_w", "m_dt_w", "m_dt_b", "m_A_log", "m_D",
          "m_out_proj_w", "fuse_w", "att_fc1_w", "att_fc1_b", "att_fc2_w",
          "att_fc2_b", "res_scale"]


def _fingerprint(inputs):
    hsh = hashlib.blake2b(digest_size=16)
    for k in _WKEYS:
        hsh.update(np.ascontiguousarray(np.asarray(inputs[k], np.float32)).tobytes())
    return hsh.hexdigest()


def _get_state(inputs):
    fp = _fingerprint(inputs)
    st = _cache.get("st")
    if st is not None and st["fp"] == fp:
        return st

    import jax
    from jax.sharding import Mesh, PartitionSpec, NamedSharding
    from jax.experimental.shard_map import shard_map
    from concourse import bass2jax

    nc = build_full(_prep_weights(inputs))
    split_excess_waits(nc)
    bass2jax.install_neuronx_cc_hook()

    devs = jax.devices()[:NCORE]
    mesh = Mesh(np.asarray(devs), ("core",))
    shard = NamedSharding(mesh, PartitionSpec("core"))
    out_avals = (jax.core.ShapedArray((C_, L), f8np),)
    Pc = PartitionSpec("core")

    def _body(x, odonor):
        outs = bass2jax._bass_exec_p.bind(
            x, odonor, bass2jax.partition_id_tensor(),
            out_avals=tuple(out_avals),
            in_names=("x", "o", "partition_id"),
            out_names=("o",),
            lowering_input_output_aliases=(),
            sim_require_finite=True,
            sim_require_nnan=True,
            nc=nc,
        )
        return tuple(outs)

    fn = jax.jit(
        shard_map(_body, mesh=mesh, in_specs=(Pc, Pc), out_specs=(Pc,),
                  check_rep=False),
        donate_argnums=(1,), keep_unused=True,
    )
    donor = jax.device_put(np.zeros((NCORE * C_, L), f8np), shard)
    # fp8 -> fp32 lookup table with the 1/OUT_SCALE fold
    lut = (np.arange(256, dtype=np.uint8).view(f8np).astype(np.float32)
           / OUT_SCALE)
    st = {"fp": fp, "fn": fn, "shard": shard, "donor": donor, "jax": jax,
          "lut": lut, "devs": devs}
    _cache["st"] = st
    return st


def kernel(**inputs):
    st = _get_state(inputs)
    jax = st["jax"]
    x = np.asarray(inputs["x"], np.float32)
    xb = x.reshape(NCORE, C_, L)
    # per-shard cast + put so shard i uploads while shard i+1 casts
    parts = [jax.device_put(xb[i].astype(f8np), st["devs"][i])
             for i in range(NCORE)]
    xd = jax.make_array_from_single_device_arrays(
        (NCORE * C_, L), st["shard"], parts)
    (out,) = st["fn"](xd, st["donor"])
    st["donor"] = out
    delta = st["lut"][np.asarray(out).view(np.uint8)]
    return x + delta.reshape(B_, C_, H_, W_)


# revision 3
# speedup vs baseline: 1.3845x; 1.3845x over previous
"""Trainium2 Bass kernel for nn_LFVSSMBlockV66 (B=4, C=128, H=W=64).

Single launch on 4 cores, one full batch per core.  All weights are baked
into the NEFF as inline constants (loaded to HBM once at model-load time),
so the only per-call tunnel traffic is x (fp8_e3m4, 0.5 MB/core) in and
the scaled residual delta (fp8_e3m4, 0.5 MB/core) out.  The host applies
the final residual add in fp32: out = x + delta / OUT_SCALE.

Device program = the previous half-split launch-1 program with the
channel-half loop unrolled (two halves of 80 scan channels processed
sequentially) plus the SE-attention tail merged in (the old launch 2).
dBu is formed inside the scan as (G5@delta)*(G5@u)*BP -- the G5 gather is
a row-copy, so gather(delta*u) == gather(delta)*gather(u); this avoids
materialising delta*u at [160 x L].

Runner: a cached jax.jit of shard_map(bass_exec) over a fixed 4-device
mesh; the output donor buffer is cycled call-to-call (the kernel fully
writes its output, so donor contents are irrelevant), avoiding the
per-call upload of zero buffers.
"""
import sys, os
sys.path.insert(0, '/opt/trn_rl_repo')

import hashlib
import numpy as np
import ml_dtypes
from contextlib import ExitStack

from concourse import bass, mybir, tile

fp32 = mybir.dt.float32
bf16 = mybir.dt.bfloat16
f8 = mybir.dt.float8e3
AF = mybir.ActivationFunctionType
OP = mybir.AluOpType

B_, C_, H_, W_ = 4, 128, 64, 64
L = H_ * W_                      # 4096
DIN, N, DTR = 160, 24, 8
DH = DIN // 2                    # 80 per half
NT = 16                          # d-tiles of 5 per half
G = C_ // 4                      # 32
CH = 512                         # phase-A chunk (one psum bank)
CHS = 1024                       # scan chunk
NCH = L // CH                    # 8
NCHS = L // CHS                  # 4
EPS = 1e-5
NCORE = 4
OUT_SCALE = 16.0

bf = ml_dtypes.bfloat16
f8np = ml_dtypes.float8_e3m4

# This container's walrus rejects >1 sync wait per instruction; split the
# extras onto NoOps.
_ws_ctr = [0]


def split_excess_waits(nc, max_waits=1):
    for fn in nc.m.functions:
        for blk in fn.blocks:
            out, changed = [], False
            for inst in blk.instructions:
                si = getattr(inst, 'sync_info', None)
                waits = list(si.on_wait) if si is not None and si.on_wait else []
                if len(waits) > max_waits:
                    for w in waits[:-max_waits]:
                        nop = mybir.InstNoOp(name=f"I-ws{_ws_ctr[0]}", ins=[], outs=[])
                        _ws_ctr[0] += 1
                        nop.engine = inst.engine
                        nop.sync_info = mybir.SyncInfo(on_wait=[w], on_update=[])
                        out.append(nop)
                    inst.sync_info = mybir.SyncInfo(
                        on_wait=waits[-max_waits:], on_update=list(si.on_update))
                    changed = True
                out.append(inst)
            if changed:
                blk.instructions = out


def _seq_views(ap2d):
    """Per-group seq-order read views of a (128, 4096) C-major spatial AP:
    v_g[c, l] = x[32g + c, pi_g(l)].  Partition-aligned (view g lives on
    partitions 32g..32g+31)."""
    v0 = ap2d[0:G, :]
    v1 = ap2d[G:2 * G, :][:, ::-1]
    v2 = ap2d[2 * G:3 * G, :].rearrange('p (h w) -> p h w', h=64).transpose([0, 2, 1])
    v3 = ap2d[3 * G:4 * G, :].rearrange('p (h w) -> p h w', h=64).transpose([0, 2, 1])[:, ::-1, ::-1]
    return [v0, v1, v2, v3]


def _chunk(view, c0, csz):
    if view.ndim == 2:
        return view[:, c0:c0 + csz]
    rows = view.shape[2]
    return view[:, c0 // rows:(c0 + csz) // rows, :]


def _f3(ap):
    """(p, csz) -> (p, csz//64, 64) to shape-match 3D chunk views."""
    return ap.rearrange('p (a b) -> p a b', b=64)


def _prep_weights(inputs):
    """Host-side inline-constant tensors (identical on all cores)."""
    bfc = lambda a: np.ascontiguousarray(np.asarray(a, dtype=np.float32)).astype(bf)
    f32c = lambda a: np.ascontiguousarray(np.asarray(a, dtype=np.float32))
    w = {}
    w["pre_g"] = f32c(inputs["pre_gamma"]).reshape(C_, 1)
    w["pre_b"] = f32c(inputs["pre_beta"]).reshape(C_, 1)
    w["gb_g"] = f32c(inputs["gb_norm_gamma"]).reshape(C_, 1)
    w["gb_b"] = f32c(inputs["gb_norm_beta"]).reshape(C_, 1)
    w["ones1"] = bfc(np.ones((1, C_)))
    w["o128"] = bfc(np.full((C_, 1), 1.0 / C_))
    w["o32"] = bfc(np.full((G, 1), 1.0 / C_))
    w["epsv"] = np.full((C_, 1), EPS, np.float32)
    w["onesr"] = bfc(np.ones((1, CH)))
    w["conv1T"] = bfc(np.asarray(inputs["lb_conv1_w"]).T)
    dwall = np.concatenate([np.asarray(inputs["lb_dw1_w"]),
                            np.asarray(inputs["lb_dw2_w"]),
                            np.asarray(inputs["lb_dw3_w"])], axis=0)
    dw9 = np.zeros((96, 9 * 96), np.float32)
    for k in range(9):
        dw9[np.arange(96), k * 96 + np.arange(96)] = dwall[:, k // 3, k % 3]
    w["dw9T"] = bfc(dw9)
    pwt = np.asarray(inputs["lb_pw_w"]).astype(np.float32).T    # (128, 128)
    w["pwAT"] = bfc(pwt[0:G, :])
    w["pwBT"] = bfc(pwt[G:, :])
    fuse = np.asarray(inputs["fuse_w"]).astype(np.float32)      # (128, 256)
    w["fuseLT"] = bfc(fuse[:, :C_].T)
    w["fuseXT"] = bfc((fuse[:, :C_] + fuse[:, C_:]).T)
    gbs = float(np.asarray(inputs["gb_scale"]).reshape(-1)[0])
    w["w2T"] = bfc((gbs * fuse[:, C_:]).T)
    fusT = np.asarray(inputs["gb_fusion_w"]).T
    w["fusGT"] = bfc(np.concatenate([fusT[gi * G:(gi + 1) * G, :]
                                     for gi in range(4)], axis=1))
    inw = np.asarray(inputs["m_in_proj_w"]).astype(np.float32)  # (320, 128)
    w["inpT"] = bfc(inw.T)                                      # (128, 320)
    cw = np.asarray(inputs["m_conv_w"]).astype(np.float32)      # (160, 4)
    cb = f32c(inputs["m_conv_b"])
    xp = np.asarray(inputs["m_x_proj_w"]).astype(np.float32)    # (56, 160)
    dtw = np.asarray(inputs["m_dt_w"]).astype(np.float32)       # (160, 8)
    A = -np.exp(np.asarray(inputs["m_A_log"], dtype=np.float32))  # (160, 24)
    ow = np.asarray(inputs["m_out_proj_w"]).astype(np.float32)  # (128, 160)
    Dp = f32c(inputs["m_D"])
    dtb = f32c(inputs["m_dt_b"])
    for h in range(2):
        o0 = h * DH
        convd = np.zeros((DH, 4 * DH), np.float32)
        for k in range(4):
            convd[np.arange(DH), k * DH + np.arange(DH)] = cw[o0:o0 + DH, k]
        w[f"convdT{h}"] = bfc(convd)
        w[f"convbT{h}"] = bfc(cb[o0:o0 + DH].reshape(1, DH))
        w[f"xpT{h}"] = bfc(xp[:, o0:o0 + DH].T)
        w[f"dtT{h}"] = bfc(dtw[o0:o0 + DH, :].T)
        w[f"dt_b{h}"] = dtb[o0:o0 + DH].reshape(DH, 1)
        A_P = np.zeros((120, NT), np.float32)
        for t in range(NT):
            for n in range(N):
                for j in range(5):
                    A_P[n * 5 + j, t] = A[o0 + t * 5 + j, n]
        w[f"A_P{h}"] = A_P
        w[f"outT{h}"] = bfc(ow[:, o0:o0 + DH].T)
        w[f"D80_{h}"] = Dp[o0:o0 + DH].reshape(DH, 1)
    R24m = np.zeros((N, 120), np.float32)
    for n in range(N):
        R24m[n, n * 5:(n + 1) * 5] = 1.0
    w["R24"] = bfc(R24m)
    G5a = np.zeros((DH, NT * 120), np.float32)
    S = np.zeros((120, NT * DH), np.float32)
    for t in range(NT):
        for n in range(N):
            for j in range(5):
                G5a[t * 5 + j, t * 120 + n * 5 + j] = 1.0
                S[n * 5 + j, t * DH + t * 5 + j] = 1.0
    w["G5all"] = bfc(G5a)
    w["S_all"] = bfc(S)
    w["fc1T"] = bfc((np.asarray(inputs["att_fc1_w"], dtype=np.float32) / L).T)
    w["b1"] = f32c(inputs["att_fc1_b"]).reshape(16, 1)
    w["fc2T"] = bfc(np.asarray(inputs["att_fc2_w"]).T)
    w["b2"] = f32c(inputs["att_fc2_b"]).reshape(C_, 1)
    rs = float(np.asarray(inputs["res_scale"]).reshape(-1)[0])
    w["resv"] = np.full((C_, 1), rs * OUT_SCALE, np.float32)
    return w


def build_full(wts, debug=False):
    nc = bass.Bass()
    x_in = nc.declare_dram_parameter("x", [C_, L], f8, isOutput=False)
    o_out = nc.declare_dram_parameter("o", [C_, L], f8, isOutput=True)
    W = {k: nc.inline_tensor(v, name=f"c_{k}") for k, v in wts.items()}
    dbg = {}
    if debug:
        P = nc.declare_dram_parameter
        for nm, shp, dt in [("d_xnb", [C_, L], bf16), ("d_seqC", [C_, L], bf16),
                            ("d_u", [2 * DH, L], bf16), ("d_dblb", [56, L], bf16),
                            ("d_del", [2 * DH, L], bf16), ("d_y3", [2 * DH, L], bf16),
                            ("d_osb", [C_, L], bf16), ("d_fgb", [C_, L], bf16),
                            ("d_local", [C_, L], bf16), ("d_fused", [C_, L], bf16),
                            ("d_pool", [C_, 1], fp32)]:
            dbg[nm] = P(nm, shp, dt, isOutput=True)

    with tile.TileContext(nc) as tc, ExitStack() as ctx:
        wp = ctx.enter_context(tc.tile_pool(name="wp", bufs=1))
        pp = ctx.enter_context(tc.tile_pool(name="pp", bufs=1))
        s2 = ctx.enter_context(tc.tile_pool(name="s2", bufs=2))

        def load(key, shape, dt):
            t = wp.tile(shape, dt, tag=f"w_{key}", name=f"w_{key}")
            nc.sync.dma_start(t[:], W[key][:])
            return t

        w_pre_g = load("pre_g", [C_, 1], fp32)
        w_pre_b = load("pre_b", [C_, 1], fp32)
        w_gb_g = load("gb_g", [C_, 1], fp32)
        w_gb_b = load("gb_b", [C_, 1], fp32)
        w_ones1 = load("ones1", [1, C_], bf16)
        w_o128 = load("o128", [C_, 1], bf16)
        w_o32 = load("o32", [G, 1], bf16)
        w_epsv = load("epsv", [C_, 1], fp32)
        w_onesr = load("onesr", [1, CH], bf16)
        w_conv1T = load("conv1T", [G, G], bf16)
        w_dw9T = load("dw9T", [96, 9 * 96], bf16)
        w_pwAT = load("pwAT", [G, C_], bf16)
        w_pwBT = load("pwBT", [96, C_], bf16)
        w_fuseLT = load("fuseLT", [C_, C_], bf16)
        w_fuseXT = load("fuseXT", [C_, C_], bf16)
        w_w2T = load("w2T", [C_, C_], bf16)
        w_fusGT = load("fusGT", [G, 4 * C_], bf16)
        w_inpT = load("inpT", [C_, 320], bf16)
        w_convdT = [load(f"convdT{h}", [DH, 4 * DH], bf16) for h in range(2)]
        w_convbT = [load(f"convbT{h}", [1, DH], bf16) for h in range(2)]
        w_xpT = [load(f"xpT{h}", [DH, 56], bf16) for h in range(2)]
        w_dtT = [load(f"dtT{h}", [DTR, DH], bf16) for h in range(2)]
        w_dt_b = [load(f"dt_b{h}", [DH, 1], fp32) for h in range(2)]
        w_A_P = [load(f"A_P{h}", [120, NT], fp32) for h in range(2)]
        w_outT = [load(f"outT{h}", [DH, C_], bf16) for h in range(2)]
        w_D80 = [load(f"D80_{h}", [DH, 1], fp32) for h in range(2)]
        w_R24 = load("R24", [N, 120], bf16)
        w_G5all = load("G5all", [DH, NT * 120], bf16)
        w_S = load("S_all", [120, NT * DH], bf16)
        w_fc1T = load("fc1T", [C_, 16], bf16)
        w_b1 = load("b1", [16, 1], fp32)
        w_fc2T = load("fc2T", [16, C_], bf16)
        w_b2 = load("b2", [C_, 1], fp32)
        w_resv = load("resv", [C_, 1], fp32)

        def ln_stats(row_pairs, rp):
            """row_pairs(kind, c0) -> [(lhsT_ap, rhs_ap)] accumulated into a
            (1, CH) stat psum.  Returns (mu_row, rs_row) (1, L) bf16 tiles."""
            murow = rp.tile([1, L], bf16, tag="murow")
            s2row = rp.tile([1, L], bf16, tag="s2row")
            rows = {'mu': murow, 's2': s2row}
            with tc.tile_pool(name="st_ps", bufs=2, space="PSUM") as stp:
                for c0 in range(0, L, CH):
                    for kind in ('mu', 's2'):
                        ps_t = stp.tile([1, CH], fp32, tag=f"ps_{kind}")
                        pairs = row_pairs(kind, c0)
                        for i, (lh, rh) in enumerate(pairs):
                            nc.tensor.matmul(ps_t[:], lh, rh, start=(i == 0),
                                             stop=(i == len(pairs) - 1))
                        nc.scalar.copy(rows[kind][:, c0:c0 + CH], ps_t[:])
            mup = s2.tile([C_, 32], bf16, tag="mup")
            nc.sync.dma_start(mup[:], murow[:])
            s2p = s2.tile([C_, 32], bf16, tag="s2p")
            nc.sync.dma_start(s2p[:], s2row[:])
            musq = s2.tile([C_, 32], fp32, tag="musq")
            nc.scalar.square(musq[:], mup[:])
            var = s2.tile([C_, 32], fp32, tag="var")
            nc.vector.tensor_sub(var[:], s2p[:], musq[:])
            sd = s2.tile([C_, 32], fp32, tag="sd")
            nc.scalar.activation(sd[:], var[:], AF.Sqrt, bias=w_epsv[:])
            rsp = s2.tile([C_, 32], fp32, tag="rsp")
            nc.vector.reciprocal(rsp[:], sd[:])
            rsbp = s2.tile([C_, 32], bf16, tag="rsbp")
            nc.vector.tensor_copy(rsbp[:], rsp[:])
            rsr = rp.tile([1, L], bf16, tag="rsr")
            nc.sync.dma_start(rsr[:], rsbp[:])
            return murow, rsr

        # ---------------- pre-LN ----------------
        xnb = pp.tile([C_, L], bf16, tag="xnb")
        localb = pp.tile([C_, L], bf16, tag="localb")
        with tc.tile_pool(name="ph1", bufs=1) as p1:
            x8t = p1.tile([C_, L], f8, tag="x8t")
            nc.sync.dma_start(x8t[:], x_in[:])
            xCb = p1.tile([C_, L], bf16, tag="xCb")
            nc.scalar.copy(xCb[:], x8t[:])

            def pre_rows(kind, c0):
                if kind == 'mu':
                    return [(w_o128[:], xCb[:, c0:c0 + CH])]
                sqs = s2.tile([C_, CH], bf16, tag="sqsP")
                nc.scalar.square(sqs[:], xCb[:, c0:c0 + CH])
                return [(w_o128[:], sqs[:])]

            mur, rsr = ln_stats(pre_rows, p1)
            with tc.tile_pool(name="bc_ps", bufs=2, space="PSUM") as bcp:
                for c0 in range(0, L, CH):
                    muP = bcp.tile([C_, CH], fp32, tag="muP")
                    nc.tensor.matmul(muP[:], w_ones1[:], mur[:, c0:c0 + CH],
                                     start=True, stop=True)
                    rsP = bcp.tile([C_, CH], fp32, tag="rsP")
                    nc.tensor.matmul(rsP[:], w_ones1[:], rsr[:, c0:c0 + CH],
                                     start=True, stop=True)
                    t1 = s2.tile([C_, CH], fp32, tag="t1")
                    nc.vector.tensor_sub(t1[:], xCb[:, c0:c0 + CH], muP[:])
                    nc.vector.tensor_mul(t1[:], t1[:], rsP[:])
                    nc.vector.tensor_scalar(xnb[:, c0:c0 + CH], t1[:], w_pre_g[:],
                                            w_pre_b[:], OP.mult, OP.add)
        if debug:
            nc.sync.dma_start(dbg["d_xnb"][:], xnb[:])

        # ---------------- local branch ----------------
        with tc.tile_pool(name="ph2", bufs=1) as p2:
            pad0 = p2.tile([96, 66 * 66], bf16, tag="pad0")
            nc.vector.memset(pad0[:], 0.0)
            pad0v = pad0[:].rearrange('p (r c) -> p r c', r=66)
            nc.sync.dma_start(pad0v[:, 1:65, 1:65],
                              xnb[G:, :].rearrange('p (h w) -> p h w', h=64))
            y_a = p2.tile([G, L], bf16, tag="y_a")
            y_bb = p2.tile([96, L], bf16, tag="y_bb")
            with tc.tile_pool(name="lb_ps", bufs=2, space="PSUM") as lbp:
                for c0 in range(0, L, CH):
                    r0 = c0 // 64
                    y32 = lbp.tile([G, CH], fp32, tag="y32")
                    nc.tensor.matmul(y32[:], w_conv1T[:], xnb[0:G, c0:c0 + CH],
                                     start=True, stop=True)
                    nc.scalar.copy(y_a[:, c0:c0 + CH], y32[:])
                    y96 = lbp.tile([96, CH], fp32, tag="y96")
                    for k in range(9):
                        ky, kx = k // 3, k % 3
                        rhs = pad0v[:, ky + r0:ky + r0 + 8, kx:kx + 64]
                        nc.tensor.matmul(y96[:], w_dw9T[:, k * 96:(k + 1) * 96],
                                         rhs, start=(k == 0), stop=(k == 8))
                    nc.scalar.copy(y_bb[:, c0:c0 + CH], y96[:])
                for c0 in range(0, L, CH):
                    pw_ps = lbp.tile([C_, CH], fp32, tag="pw_ps")
                    nc.tensor.matmul(pw_ps[:], w_pwAT[:], y_a[:, c0:c0 + CH],
                                     start=True, stop=False)
                    nc.tensor.matmul(pw_ps[:], w_pwBT[:], y_bb[:, c0:c0 + CH],
                                     start=False, stop=True)
                    lr1 = s2.tile([C_, CH], bf16, tag="lr1")
                    nc.vector.tensor_scalar(lr1[:], pw_ps[:], 0.1, None, OP.mult)
                    nc.vector.tensor_tensor(localb[:, c0:c0 + CH], pw_ps[:], lr1[:],
                                            OP.max)
        if debug:
            nc.sync.dma_start(dbg["d_local"][:], localb[:])

        with tc.tile_pool(name="p_tail", bufs=1) as p_tail, \
             tc.tile_pool(name="p_y3", bufs=1) as p_y3:
          with tc.tile_pool(name="p_scan", bufs=1) as p_scan:
            # ---------------- gb-LN + seq build ----------------
            with tc.tile_pool(name="p_seq", bufs=1) as p_seq:
                seqC = p_seq.tile([C_, L], bf16, tag="seqC")
                with tc.tile_pool(name="ph3", bufs=1) as p3:
                    xn1 = p3.tile([G, L], bf16, tag="xn1")
                    nc.sync.dma_start(xn1[:], xnb[G:2 * G, :])
                    xn2 = p3.tile([G, L], bf16, tag="xn2")
                    nc.sync.dma_start(xn2[:], xnb[2 * G:3 * G, :])
                    xn3 = p3.tile([G, L], bf16, tag="xn3")
                    nc.sync.dma_start(xn3[:], xnb[3 * G:, :])

                    def g_view(t, gi):
                        if gi == 1:
                            return t[:][:, ::-1]
                        v = t[:].rearrange('p (h w) -> p h w', h=64).transpose([0, 2, 1])
                        return v if gi == 2 else v[:, ::-1, ::-1]

                    # copies (partition 0-31 based) for the squares / stats
                    xnv = [xnb[0:G, :]] + [g_view(t, gi + 1)
                                           for gi, t in enumerate((xn1, xn2, xn3))]

                    def gb_rows(kind, c0):
                        if kind == 'mu':
                            return [(w_o32[:], _chunk(xnv[gi], c0, CH))
                                    for gi in range(4)]
                        pairs = []
                        for gi in range(4):
                            sqs = s2.tile([G, CH], bf16, tag="sqsP")
                            srcv = _chunk(xnv[gi], c0, CH)
                            nc.scalar.square(
                                _f3(sqs[:]) if srcv.ndim == 3 else sqs[:], srcv)
                            pairs.append((w_o32[:], sqs[:]))
                        return pairs

                    mur2, rsr2 = ln_stats(gb_rows, p3)

                    # partition-aligned direct views for the normalisation
                    xnv_t = _seq_views(xnb[:])
                    with tc.tile_pool(name="bc2_ps", bufs=2, space="PSUM") as bcp:
                        for c0 in range(0, L, CH):
                            muP = bcp.tile([C_, CH], fp32, tag="muP2")
                            nc.tensor.matmul(muP[:], w_ones1[:], mur2[:, c0:c0 + CH],
                                             start=True, stop=True)
                            rsP = bcp.tile([C_, CH], fp32, tag="rsP2")
                            nc.tensor.matmul(rsP[:], w_ones1[:], rsr2[:, c0:c0 + CH],
                                             start=True, stop=True)
                            tg = s2.tile([C_, CH], fp32, tag="tg")
                            for gi in range(4):
                                srcv = _chunk(xnv_t[gi], c0, CH)
                                sl = slice(gi * G, (gi + 1) * G)
                                if srcv.ndim == 3:
                                    nc.vector.tensor_sub(_f3(tg[sl, :]), srcv,
                                                         _f3(muP[sl, :]))
                                else:
                                    nc.vector.tensor_sub(tg[sl, :], srcv, muP[sl, :])
                                nc.vector.tensor_mul(tg[sl, :], tg[sl, :], rsP[sl, :])
                            nc.vector.tensor_scalar(seqC[:, c0:c0 + CH], tg[:],
                                                    w_gb_g[:], w_gb_b[:],
                                                    OP.mult, OP.add)
                if debug:
                    nc.sync.dma_start(dbg["d_seqC"][:], seqC[:])

                # ------------- in_proj + conv1d + silu -------------
                zs = [p_scan.tile([DH, L], bf16, tag=f"zs{h}", name=f"zs{h}")
                      for h in range(2)]
                u_t = [p_scan.tile([DH, L], bf16, tag=f"u{h}", name=f"u{h}")
                       for h in range(2)]
                xr_pad = [p_seq.tile([DH, L + 3], bf16, tag=f"xrp{h}",
                                     name=f"xrp{h}") for h in range(2)]
                for h in range(2):
                    nc.vector.memset(xr_pad[h][:, 0:3], 0.0)
                with tc.tile_pool(name="ip_ps", bufs=2, space="PSUM") as ipp:
                    for c0 in range(0, L, CH):
                        for h in range(2):
                            xr_ps = ipp.tile([DH, CH], fp32, tag=f"xr_ps{h}",
                                             name=f"xr_ps{h}")
                            nc.tensor.matmul(xr_ps[:], w_inpT[:, h * DH:(h + 1) * DH],
                                             seqC[:, c0:c0 + CH], start=True, stop=True)
                            nc.scalar.copy(xr_pad[h][:, 3 + c0:3 + c0 + CH], xr_ps[:])
                            z_ps = ipp.tile([DH, CH], fp32, tag=f"z_ps{h}",
                                            name=f"z_ps{h}")
                            nc.tensor.matmul(z_ps[:],
                                             w_inpT[:, 160 + h * DH:160 + (h + 1) * DH],
                                             seqC[:, c0:c0 + CH], start=True, stop=True)
                            zsg = s2.tile([DH, CH], bf16, tag="zsg")
                            nc.scalar.activation(zsg[:], z_ps[:], AF.Sigmoid)
                            nc.vector.tensor_mul(zs[h][:, c0:c0 + CH], zsg[:], z_ps[:])

                with tc.tile_pool(name="cv_ps", bufs=2, space="PSUM") as cvp:
                    for c0 in range(0, L, CH):
                        for h in range(2):
                            cv_ps = cvp.tile([DH, CH], fp32, tag="cv_ps")
                            for k in range(4):
                                nc.tensor.matmul(
                                    cv_ps[:],
                                    w_convdT[h][:, k * DH:(k + 1) * DH],
                                    xr_pad[h][:, c0 + k:c0 + k + CH],
                                    start=(k == 0), stop=False)
                            nc.tensor.matmul(cv_ps[:], w_convbT[h][:], w_onesr[:],
                                             start=False, stop=True)
                            usg2 = s2.tile([DH, CH], bf16, tag="usg2")
                            nc.scalar.activation(usg2[:], cv_ps[:], AF.Sigmoid)
                            nc.vector.tensor_mul(u_t[h][:, c0:c0 + CH], usg2[:],
                                                 cv_ps[:])
            if debug:
                nc.sync.dma_start(dbg["d_u"][0:DH, :], u_t[0][:])
                nc.sync.dma_start(dbg["d_u"][DH:, :], u_t[1][:])

            # ------------- x_proj + delta + B/C spread -------------
            del_t = [p_scan.tile([DH, L], bf16, tag=f"del{h}", name=f"del{h}")
                     for h in range(2)]
            BP = p_scan.tile([120, L], bf16, tag="BP")
            CPt = p_scan.tile([120, L], bf16, tag="CPt")
            with tc.tile_pool(name="p_dbc", bufs=1) as p_dbc:
                dtc = p_dbc.tile([DTR, L], bf16, tag="dtc")
                Bc = p_dbc.tile([N, L], bf16, tag="Bc")
                Cc = p_dbc.tile([N, L], bf16, tag="Cc")
                with tc.tile_pool(name="xp_ps", bufs=2, space="PSUM") as xpp:
                    for c0 in range(0, L, CH):
                        for nm, dst, lo, hi in (("dt_o", dtc, 0, DTR),
                                                ("b_o", Bc, DTR, DTR + N),
                                                ("c_o", Cc, DTR + N, 56)):
                            o_ps = xpp.tile([hi - lo, CH], fp32, tag=nm, name=nm)
                            for h in range(2):
                                nc.tensor.matmul(
                                    o_ps[:], w_xpT[h][:, lo:hi],
                                    u_t[h][:, c0:c0 + CH],
                                    start=(h == 0), stop=(h == 1))
                            nc.scalar.copy(dst[:, c0:c0 + CH], o_ps[:])
                if debug:
                    nc.sync.dma_start(dbg["d_dblb"][0:DTR, :], dtc[:])
                    nc.sync.dma_start(dbg["d_dblb"][DTR:DTR + N, :], Bc[:])
                    nc.sync.dma_start(dbg["d_dblb"][DTR + N:, :], Cc[:])

                # delta per half: softplus via exp+ln
                e80 = p_dbc.tile([DH, L], bf16, tag="e80")
                for h in range(2):
                    with tc.tile_pool(name="dt_ps", bufs=2, space="PSUM") as dtp:
                        for c0 in range(0, L, CH):
                            dt_ps = dtp.tile([DH, CH], fp32, tag="dt_ps")
                            nc.tensor.matmul(dt_ps[:], w_dtT[h][:],
                                             dtc[:, c0:c0 + CH],
                                             start=True, stop=True)
                            nc.scalar.activation(e80[:, c0:c0 + CH], dt_ps[:],
                                                 AF.Exp, bias=w_dt_b[h][:])
                    nc.scalar.activation(del_t[h][:], e80[:], AF.Ln, bias=1.0)
                if debug:
                    nc.sync.dma_start(dbg["d_del"][0:DH, :], del_t[0][:])
                    nc.sync.dma_start(dbg["d_del"][DH:, :], del_t[1][:])

                with tc.tile_pool(name="bc3_ps", bufs=2, space="PSUM") as bcp:
                    for c0 in range(0, L, CH):
                        bp_ps = bcp.tile([120, CH], fp32, tag="bp_ps")
                        nc.tensor.matmul(bp_ps[:], w_R24[:], Bc[:, c0:c0 + CH],
                                         start=True, stop=True)
                        nc.scalar.copy(BP[:, c0:c0 + CH], bp_ps[:])
                        cp_ps = bcp.tile([120, CH], fp32, tag="cp_ps")
                        nc.tensor.matmul(cp_ps[:], w_R24[:], Cc[:, c0:c0 + CH],
                                         start=True, stop=True)
                        nc.scalar.copy(CPt[:, c0:c0 + CH], cp_ps[:])

            # ---------------- selective scan (both halves) ----------------
            y3 = [p_y3.tile([DH, L], bf16, tag=f"y3_{h}", name=f"y3_{h}")
                  for h in range(2)]
            hstate = pp.tile([120, 2 * NT], bf16, tag="hstate")
            with tc.tile_pool(name="sc_ps", bufs=2, space="PSUM") as scp, \
                 tc.tile_pool(name="scu_ps", bufs=1, space="PSUM") as scup, \
                 tc.tile_pool(name="scy_ps", bufs=1, space="PSUM") as scyp, \
                 tc.tile_pool(name="scs", bufs=2) as scs:
                for h in range(2):
                    for ci in range(NCHS):
                        c0 = ci * CHS
                        y_ps = scyp.tile([DH, CHS], fp32, tag="y_ps")
                        for t in range(NT):
                            ts_ = h * NT + t
                            dP = scp.tile([120, CHS], fp32, tag="dP")
                            for s in range(2):
                                nc.tensor.matmul(
                                    dP[:, s * CH:(s + 1) * CH],
                                    w_G5all[:, t * 120:(t + 1) * 120],
                                    del_t[h][:, c0 + s * CH:c0 + (s + 1) * CH],
                                    start=True, stop=True)
                            dA = scs.tile([120, CHS], fp32, tag="dA")
                            nc.scalar.activation(dA[:], dP[:], AF.Exp,
                                                 scale=w_A_P[h][:, t:t + 1])
                            uP = scup.tile([120, CHS], fp32, tag="uP")
                            for s in range(2):
                                nc.tensor.matmul(
                                    uP[:, s * CH:(s + 1) * CH],
                                    w_G5all[:, t * 120:(t + 1) * 120],
                                    u_t[h][:, c0 + s * CH:c0 + (s + 1) * CH],
                                    start=True, stop=True)
                            uB = scs.tile([120, CHS], bf16, tag="uB")
                            nc.vector.tensor_mul(uB[:], uP[:], BP[:, c0:c0 + CHS])
                            dBu = scs.tile([120, CHS], bf16, tag="dBu")
                            nc.vector.tensor_mul(dBu[:], uB[:], dP[:])
                            hh = scs.tile([120, CHS], bf16, tag="hh")
                            init = 0.0 if ci == 0 else hstate[:, ts_:ts_ + 1]
                            nc.vector.tensor_tensor_scan(hh[:], dA[:], dBu[:], init,
                                                         OP.mult, OP.add)
                            nc.vector.tensor_copy(hstate[:, ts_:ts_ + 1],
                                                  hh[:, CHS - 1:CHS])
                            hC = scs.tile([120, CHS], bf16, tag="hC")
                            nc.vector.tensor_mul(hC[:], hh[:], CPt[:, c0:c0 + CHS])
                            for s in range(2):
                                nc.tensor.matmul(y_ps[:, s * CH:(s + 1) * CH],
                                                 w_S[:, t * DH:(t + 1) * DH],
                                                 hC[:, s * CH:(s + 1) * CH],
                                                 start=(t == 0), stop=(t == NT - 1))
                        y2 = scs.tile([DH, CHS], bf16, tag="y2")
                        nc.vector.scalar_tensor_tensor(y2[:], u_t[h][:, c0:c0 + CHS],
                                                       w_D80[h][:], y_ps[:],
                                                       OP.mult, OP.add)
                        nc.vector.tensor_mul(y3[h][:, c0:c0 + CHS], y2[:],
                                             zs[h][:, c0:c0 + CHS])
            if debug:
                nc.sync.dma_start(dbg["d_y3"][0:DH, :], y3[0][:])
                nc.sync.dma_start(dbg["d_y3"][DH:, :], y3[1][:])
          # ---------- out_proj (p_scan closed; y3 + osb alive) ----------
          osb = p_tail.tile([C_, L], bf16, tag="osb")
          with tc.tile_pool(name="op_ps", bufs=2, space="PSUM") as opp:
              for c0 in range(0, L, CH):
                  os_ps = opp.tile([C_, CH], fp32, tag="os_ps")
                  nc.tensor.matmul(os_ps[:], w_outT[0][:], y3[0][:, c0:c0 + CH],
                                   start=True, stop=False)
                  nc.tensor.matmul(os_ps[:], w_outT[1][:], y3[1][:, c0:c0 + CH],
                                   start=False, stop=True)
                  nc.scalar.copy(osb[:, c0:c0 + CH], os_ps[:])
          if debug:
              nc.sync.dma_start(dbg["d_osb"][:], osb[:])

          # ---------- un-scan + fusion ----------
        # (indent note: still inside the p_tail/p_y3 with-block)
        if True:
            fgb = p_tail.tile([C_, L], bf16, tag="fgb")
            with tc.tile_pool(name="ph6", bufs=1) as p6:
                os1c = p6.tile([G, L], bf16, tag="os1c")
                nc.sync.dma_start(os1c[:], osb[G:2 * G, :])
                os2c = p6.tile([G, L], bf16, tag="os2c")
                nc.sync.dma_start(os2c[:], osb[2 * G:3 * G, :])
                os3 = p6.tile([G, L], bf16, tag="os3")
                nc.sync.dma_start(os3[:], osb[3 * G:, :])

                def r_view(t, gi):
                    if gi == 1:
                        return t[:][:, ::-1]
                    v = t[:].rearrange('p (w h) -> p w h', w=64).transpose([0, 2, 1])
                    return v if gi == 2 else v[:, ::-1, ::-1]

                rvs = [osb[0:G, :], r_view(os1c, 1), r_view(os2c, 2), r_view(os3, 3)]
                with tc.tile_pool(name="fg_ps", bufs=2, space="PSUM") as fgp:
                    for c0 in range(0, L, CH):
                        fg_ps = fgp.tile([C_, CH], fp32, tag="fg_ps")
                        for gi in range(4):
                            nc.tensor.matmul(fg_ps[:],
                                             w_fusGT[:, gi * C_:(gi + 1) * C_],
                                             _chunk(rvs[gi], c0, CH),
                                             start=(gi == 0), stop=(gi == 3))
                        nc.scalar.copy(fgb[:, c0:c0 + CH], fg_ps[:])
            if debug:
                nc.sync.dma_start(dbg["d_fgb"][:], fgb[:])

            # ---------- fuse + pool + SE + output ----------
            fusedb = p_tail.tile([C_, L], bf16, tag="fusedb")
            poolacc = pp.tile([C_, NCH], fp32, tag="poolacc")
            with tc.tile_pool(name="fu_ps", bufs=2, space="PSUM") as fup:
                for idx, c0 in enumerate(range(0, L, CH)):
                    fu_ps = fup.tile([C_, CH], fp32, tag="fu_ps")
                    nc.tensor.matmul(fu_ps[:], w_fuseLT[:], localb[:, c0:c0 + CH],
                                     start=True, stop=False)
                    nc.tensor.matmul(fu_ps[:], w_fuseXT[:], xnb[:, c0:c0 + CH],
                                     start=False, stop=False)
                    nc.tensor.matmul(fu_ps[:], w_w2T[:], fgb[:, c0:c0 + CH],
                                     start=False, stop=True)
                    nc.scalar.activation(fusedb[:, c0:c0 + CH], fu_ps[:], AF.Copy,
                                         accum_out=poolacc[:, idx:idx + 1])
            if debug:
                nc.sync.dma_start(dbg["d_fused"][:], fusedb[:])

            poolp = pp.tile([C_, 1], fp32, tag="poolp")
            nc.vector.tensor_reduce(poolp[:], poolacc[:], mybir.AxisListType.X,
                                    OP.add)
            if debug:
                nc.sync.dma_start(dbg["d_pool"][:], poolp[:])
            poolb = pp.tile([C_, 1], bf16, tag="poolb")
            nc.vector.tensor_copy(poolb[:], poolp[:])
            with tc.tile_pool(name="se_ps", bufs=1, space="PSUM") as sep:
                h1 = sep.tile([16, 1], fp32, tag="h1")
                nc.tensor.matmul(h1[:], w_fc1T[:], poolb[:], start=True, stop=True)
                r1 = pp.tile([16, 1], bf16, tag="r1")
                nc.scalar.activation(r1[:], h1[:], AF.Relu, bias=w_b1[:])
                a_ps = sep.tile([C_, 1], fp32, tag="a_ps")
                nc.tensor.matmul(a_ps[:], w_fc2T[:], r1[:], start=True, stop=True)
                a_t = pp.tile([C_, 1], fp32, tag="a_t")
                nc.scalar.activation(a_t[:], a_ps[:], AF.Sigmoid, bias=w_b2[:])
            s_t = pp.tile([C_, 1], fp32, tag="s_t")
            nc.vector.tensor_mul(s_t[:], a_t[:], w_resv[:])
            o8 = p_tail.tile([C_, L], f8, tag="o8")
            nc.vector.tensor_scalar(o8[:], fusedb[:], s_t[:], None, OP.mult)
            nc.sync.dma_start(o_out[:], o8[:])
    return nc


# ---------------------------------------------------------------------------
_cache = {}

_WKEYS = ["pre_gamma", "pre_beta", "lb_conv1_w", "lb_dw1_w", "lb_dw2_w",
          "lb_dw3_w", "lb_pw_w", "gb_norm_gamma", "gb_norm_beta",
          "gb_fusion_w", "gb_scale", "m_in_proj_w", "m_conv_w", "m_conv_b",
          "m_x_proj_w", "m_dt_w", "m_dt_b", "m_A_log", "m_D",
          "m_out_proj_w", "fuse_w", "att_fc1_w", "att_fc1_b", "att_fc2_w",
          "att_fc2_b", "res_scale"]


def _fingerprint(inputs):
    hsh = hashlib.blake2b(digest_size=16)
    for k in _WKEYS:
        hsh.update(np.ascontiguousarray(np.asarray(inputs[k], np.float32)).tobytes())
    return hsh.hexdigest()


def _get_state(inputs):
    fp = _fingerprint(inputs)
    st = _cache.get("st")
    if st is not None and st["fp"] == fp:
        return st

    import jax
    from jax.sharding import Mesh, PartitionSpec, NamedSharding
    from jax.experimental.shard_map import shard_map
    from concourse import bass2jax

    nc = build_full(_prep_weights(inputs))
    split_excess_waits(nc)
    bass2jax.install_neuronx_cc_hook()

    devs = jax.devices()[:NCORE]
    mesh = Mesh(np.asarray(devs), ("core",))
    shard = NamedSharding(mesh, PartitionSpec("core"))
    out_avals = (jax.core.ShapedArray((C_, L), f8np),)
    Pc = PartitionSpec("core")

    def _body(x, odonor):
        outs = bass2jax._bass_exec_p.bind(
            x, odonor, bass2jax.partition_id_tensor(),
            out_avals=tuple(out_avals),
            in_names=("x", "o", "partition_id"),
            out_names=("o",),
            lowering_input_output_aliases=(),
            sim_require_finite=True,
            sim_require_nnan=True,
            nc=nc,
        )
        return tuple(outs)

    fn = jax.jit(
        shard_map(_body, mesh=mesh, in_specs=(Pc, Pc), out_specs=(Pc,),
                  check_rep=False),
        donate_argnums=(1,), keep_unused=True,
    )
    donor = jax.device_put(np.zeros((NCORE * C_, L), f8np), shard)
    # fp8 -> fp32 lookup table with the 1/OUT_SCALE fold
    lut = (np.arange(256, dtype=np.uint8).view(f8np).astype(np.float32)
           / OUT_SCALE)
    st = {"fp": fp, "fn": fn, "shard": shard, "donor": donor, "jax": jax,
          "lut": lut, "devs": devs}
    _cache["st"] = st
    return st


def kernel(**inputs):
    st = _get_state(inputs)
    jax = st["jax"]
    x = np.asarray(inputs["x"], np.float32)
    xb = x.reshape(NCORE, C_, L)
    # per-shard cast + put so shard i uploads while shard i+1 casts
    parts = [jax.device_put(xb[i].astype(f8np), st["devs"][i])
             for i in range(NCORE)]
    xd = jax.make_array_from_single_device_arrays(
        (NCORE * C_, L), st["shard"], parts)
    (out,) = st["fn"](xd, st["donor"])
    st["donor"] = out
    out.copy_to_host_async()
    # fetch shard-by-shard so batch i's LUT decode + residual add overlaps
    # the wire transfer of batch i+1
    res = np.empty((B_, C_, H_, W_), np.float32)
    x4 = x.reshape(B_, C_, H_ * W_)
    shards = sorted(out.addressable_shards,
                    key=lambda s: (s.index[0].start or 0))
    for i, s in enumerate(shards):
        raw = np.asarray(s.data)
        np.add(x4[i], st["lut"][raw.view(np.uint8)], out=res[i].reshape(C_, L))
    return res
